# revision 7
# baseline (speedup 1.0000x reference)
"""Single-head attention (B=4, S=4096, D=512, H=64) on 8 TRN2 NeuronCores.

Sharding: core c handles batch b=c//2 and key-half h=c%2 (2048 keys), for ALL
4096 queries of that batch. Softmax uses a constant shift (exp(s/8 - 4)) so
per-key-half partial numerators/denominators are directly addable; the host
merges the two halves per batch and normalizes.

Device layout (per core):
  xt  [512, 4096] fp16  -- x[b]^T with columns rotated so the core's key half
                           is columns 0..2047 (queries therefore permuted too)
  scores^T = k_chunk @ q^T via PE (contraction over H=64 on partitions, 2-way
  row tiling), exp on ACT (PSUM->SBUF fp16), then out^T = [v|1]^T @ P^T
  accumulated over key chunks in PSUM. Output [65, 4096] f32: rows 0..63 are
  the unnormalized attention numerator^T, row 64 the softmax denominator.
"""

import numpy as np

import concourse.bass as bass
import concourse.tile as tile
import concourse.mybir as mybir
from concourse import bass_utils

B, S, D, H = 4, 4096, 512, 64
N_CORES = 8
KC = S // 2          # keys per core
NCHUNK = KC // 128   # 16 key chunks of 128
NPAIR = NCHUNK // 2  # 8 chunk pairs (2-way PE row tiling)
MT = 512             # query tile width
NM = S // MT         # 8 query tiles
VW = H + 1           # v columns + ones column
EXP_SHIFT = -4.0     # constant softmax shift; cancels in normalization

f16 = mybir.dt.float16
f32 = mybir.dt.float32


def _split_multi_waits(nc):
    """This walrus build encodes at most ONE sync-wait command per
    instruction. Hoist surplus waits onto standalone single-wait
    EventSemaphore instructions inserted just before the original."""
    ctr = 0
    for f in nc.m.functions:
        for bb in f.blocks:
            insts = bb.instructions
            i = 0
            while i < len(insts):
                inst = insts[i]
                si = inst.sync_info
                if si is not None and si.on_wait and len(si.on_wait) > 1:
                    waits = list(si.on_wait)
                    inst.sync_info = mybir.SyncInfo(
                        on_wait=[waits[-1]], on_update=list(si.on_update or [])
                    )
                    for w in waits[:-1]:
                        ev = mybir.InstEventSemaphore(
                            name=f"W-split-{ctr}", ins=[], outs=[]
                        )
                        ctr += 1
                        ev.engine = inst.engine
                        ev.sync_info = mybir.SyncInfo(on_wait=[w], on_update=[])
                        insts.insert(i, ev)
                        i += 1
                i += 1
    return ctr


def _build_nc():
    nc = bass.Bass("TRN2", target_bir_lowering=False, debug=False)
    xt = nc.dram_tensor("xt", [D, S], f16, kind="ExternalInput").ap()
    wq = nc.dram_tensor("wq", [D, 128], f16, kind="ExternalInput").ap()
    wk = nc.dram_tensor("wk", [D, 128], f16, kind="ExternalInput").ap()
    wv = nc.dram_tensor("wv", [D, VW], f16, kind="ExternalInput").ap()
    bq = nc.dram_tensor("bq", [128, 1], f32, kind="ExternalInput").ap()
    bk = nc.dram_tensor("bk", [128, 1], f32, kind="ExternalInput").ap()
    bv = nc.dram_tensor("bv", [1, VW], f32, kind="ExternalInput").ap()
    out = nc.dram_tensor("out", [VW, S], f32, kind="ExternalOutput").ap()

    NDC = D // 128  # 4 contraction chunks

    with tile.TileContext(nc) as tc:
        with (
            tc.tile_pool(name="consts", bufs=1) as consts,
            tc.tile_pool(name="xsb", bufs=1) as xsb,
            tc.tile_pool(name="qkv", bufs=1) as qkv,
            tc.tile_pool(name="pt", bufs=3) as ptp,
            tc.tile_pool(name="outsb", bufs=2) as outsb,
        ):
            # --- constants ---
            wq_sb = consts.tile([128, NDC, 128], f16)
            wk_sb = consts.tile([128, NDC, 128], f16)
            wv_sb = consts.tile([128, NDC, VW], f16)
            nc.sync.dma_start(out=wq_sb, in_=wq.rearrange("(c p) m -> p c m", p=128))
            nc.sync.dma_start(out=wk_sb, in_=wk.rearrange("(c p) m -> p c m", p=128))
            nc.sync.dma_start(out=wv_sb, in_=wv.rearrange("(c p) m -> p c m", p=128))
            bq_sb = consts.tile([128, 1], f32)
            bk_sb = consts.tile([128, 1], f32)
            bv_sb = consts.tile([128, VW], f32)
            nc.sync.dma_start(out=bq_sb, in_=bq)
            nc.sync.dma_start(out=bk_sb, in_=bk)
            bv_bcast = bass.AP(tensor=bv.tensor, offset=bv.offset, ap=[[0, 128], [1, VW]])
            nc.sync.dma_start(out=bv_sb, in_=bv_bcast)
            shift_sb = consts.tile([128, 1], f32)
            nc.vector.memset(shift_sb, EXP_SHIFT)

            # --- x^T load: 4 chunks of [128, S] ---
            x_sb = xsb.tile([128, NDC, S], f16)
            xt_r = xt.rearrange("(c p) s -> p c s", p=128)
            for c in range(NDC):
                nc.sync.dma_start(out=x_sb[:, c, :], in_=xt_r[:, c, :])

            # --- projections ---
            qTd_sb = qkv.tile([128, S], f16)      # q^T duplicated on partitions 64..127
            kTd_sb = qkv.tile([128, KC], f16)     # k^T (core's keys) duplicated
            v_sb = qkv.tile([128, NCHUNK * VW], f16)  # [v|1] natural, chunk-packed

            with tc.tile_pool(name="pproj", bufs=2, space="PSUM") as pproj:
                for m in range(NM):
                    ps = pproj.tile([128, MT], f32, tag="pq")
                    for c in range(NDC):
                        nc.tensor.matmul(
                            ps, lhsT=wq_sb[:, c, :], rhs=x_sb[:, c, m * MT:(m + 1) * MT],
                            start=(c == 0), stop=(c == NDC - 1),
                        )
                    nc.vector.tensor_scalar_add(
                        out=qTd_sb[:, m * MT:(m + 1) * MT], in0=ps, scalar1=bq_sb
                    )
                for n in range(KC // MT):
                    ps = pproj.tile([128, MT], f32, tag="pq")
                    for c in range(NDC):
                        nc.tensor.matmul(
                            ps, lhsT=wk_sb[:, c, :], rhs=x_sb[:, c, n * MT:(n + 1) * MT],
                            start=(c == 0), stop=(c == NDC - 1),
                        )
                    nc.vector.tensor_scalar_add(
                        out=kTd_sb[:, n * MT:(n + 1) * MT], in0=ps, scalar1=bk_sb
                    )
                for s_ in range(NCHUNK):
                    ps = pproj.tile([128, VW], f32, tag="pv")
                    for c in range(NDC):
                        nc.tensor.matmul(
                            ps, lhsT=x_sb[:, c, s_ * 128:(s_ + 1) * 128], rhs=wv_sb[:, c, :],
                            start=(c == 0), stop=(c == NDC - 1),
                        )
                    nc.vector.tensor_add(
                        out=v_sb[:, s_ * VW:(s_ + 1) * VW], in0=ps, in1=bv_sb
                    )

            # --- main attention loop ---
            with (
                tc.tile_pool(name="psc", bufs=3, space="PSUM") as pscp,
                tc.tile_pool(name="pout", bufs=2, space="PSUM") as poutp,
            ):
                for m in range(NM):
                    ms = m * MT
                    po = poutp.tile([VW, MT], f32)
                    pending = None  # (pt_tile, j) software pipeline: attn lags by 1
                    for j in range(NPAIR):
                        ca, cb = 2 * j, 2 * j + 1
                        psc = pscp.tile([128, 2 * MT], f32)
                        nc.tensor.matmul(
                            psc[:, 0:MT],
                            lhsT=kTd_sb[0:64, ca * 128:(ca + 1) * 128],
                            rhs=qTd_sb[0:64, ms:ms + MT],
                            start=True, stop=True,
                        )
                        nc.tensor.matmul(
                            psc[:, MT:2 * MT],
                            lhsT=kTd_sb[64:128, cb * 128:(cb + 1) * 128],
                            rhs=qTd_sb[64:128, ms:ms + MT],
                            start=True, stop=True,
                        )
                        pt = ptp.tile([128, 2 * MT], f16)
                        nc.scalar.activation(
                            out=pt, in_=psc, func=mybir.ActivationFunctionType.Exp,
                            bias=shift_sb, scale=0.125,
                        )
                        if pending is not None:
                            _emit_attn(nc, po, v_sb, pending[0], pending[1], NPAIR)
                        pending = (pt, j)
                    _emit_attn(nc, po, v_sb, pending[0], pending[1], NPAIR)
                    po_sb = outsb.tile([VW, MT], f32)
                    nc.vector.tensor_copy(out=po_sb, in_=po)
                    nc.sync.dma_start(out=out[:, ms:ms + MT], in_=po_sb)

    _split_multi_waits(nc)
    return nc


def _emit_attn(nc, po, v_sb, pt, j, npair):
    ca, cb = 2 * j, 2 * j + 1
    MT_ = pt.shape[-1] // 2
    nc.tensor.matmul(
        po, lhsT=v_sb[:, ca * VW:(ca + 1) * VW], rhs=pt[:, 0:MT_],
        start=(j == 0), stop=False,
    )
    nc.tensor.matmul(
        po, lhsT=v_sb[:, cb * VW:(cb + 1) * VW], rhs=pt[:, MT_:2 * MT_],
        start=False, stop=(j == npair - 1),
    )


_NC_CACHE = []


def _prepare_in_maps(x, Wq, bq, Wk, bk, Wv, bv):
    x = np.asarray(x, dtype=np.float32)
    Wq, Wk, Wv = (np.asarray(a, dtype=np.float32) for a in (Wq, Wk, Wv))
    bq, bk, bv = (np.asarray(a, dtype=np.float32) for a in (bq, bk, bv))

    wq_dup = np.concatenate([Wq, Wq], axis=1).astype(np.float16)      # [512,128]
    wk_dup = np.concatenate([Wk, Wk], axis=1).astype(np.float16)
    wv_aug = np.concatenate([Wv, np.zeros((D, 1), np.float32)], axis=1).astype(np.float16)
    bq_dup = np.concatenate([bq, bq]).astype(np.float32).reshape(128, 1)
    bk_dup = np.concatenate([bk, bk]).astype(np.float32).reshape(128, 1)
    bv_aug = np.concatenate([bv, np.ones(1, np.float32)]).astype(np.float32).reshape(1, VW)

    in_maps = []
    for c in range(N_CORES):
        b, h = c // 2, c % 2
        xt_b = np.ascontiguousarray(x[b].T)  # [512, 4096]
        if h == 1:
            xt_b = np.roll(xt_b, -KC, axis=1)  # core's key half first
        in_maps.append({
            "xt": xt_b.astype(np.float16),
            "wq": wq_dup, "wk": wk_dup, "wv": wv_aug,
            "bq": bq_dup, "bk": bk_dup, "bv": bv_aug,
        })
    return in_maps


def _merge_results(results):
    out = np.empty((B, S, H), dtype=np.float32)
    for b in range(B):
        a = results[2 * b]["out"].astype(np.float64)       # natural q order
        bb = results[2 * b + 1]["out"].astype(np.float64)  # q order rolled by -KC
        bb = np.roll(bb, KC, axis=1)                       # undo the roll
        tot = a + bb
        out[b] = (tot[:H, :] / tot[H:H + 1, :]).T.astype(np.float32)
    return out


def kernel(x, Wq, bq, Wk, bk, Wv, bv):
    in_maps = _prepare_in_maps(x, Wq, bq, Wk, bk, Wv, bv)
    if not _NC_CACHE:
        _NC_CACHE.append(_build_nc())
    nc = _NC_CACHE[0]
    res = bass_utils.run_bass_kernel_spmd(nc, in_maps, core_ids=list(range(N_CORES)))
    return _merge_results(res.results)


# revision 11
# speedup vs baseline: 9.9658x; 9.9658x over previous
"""Single-head attention (B=4, S=4096, D=512, H=64) on 8 TRN2 NeuronCores.

Sharding: core c handles batch b=c//2 and key-half h=c%2 (2048 keys), for ALL
4096 queries of that batch. Softmax uses a constant shift (exp(s/8 - 4)) so
per-key-half partial numerators/denominators are directly addable; the host
merges the two halves per batch and normalizes.

Device layout (per core):
  xt  [512, 4096] fp16  -- x[b]^T with columns rotated so the core's key half
                           is columns 0..2047 (queries therefore permuted too)
  scores^T = k_chunk @ q^T via PE (contraction over H=64 on partitions, 2-way
  row tiling), exp on ACT (PSUM->SBUF fp16), then out^T = [v|1]^T @ P^T
  accumulated over key chunks in PSUM. Output [65, 4096] f32: rows 0..63 are
  the unnormalized attention numerator^T, row 64 the softmax denominator.
"""

import numpy as np

import concourse.bass as bass
import concourse.tile as tile
import concourse.mybir as mybir
from concourse import bass_utils

B, S, D, H = 4, 4096, 512, 64
N_CORES = 8
KC = S // 2          # keys per core
NCHUNK = KC // 128   # 16 key chunks of 128
NPAIR = NCHUNK // 2  # 8 chunk pairs (2-way PE row tiling)
MT = 512             # query tile width
NM = S // MT         # 8 query tiles
VW = H + 1           # v columns + ones column
EXP_SHIFT = -4.0     # constant softmax shift; cancels in normalization

f16 = mybir.dt.float16
f32 = mybir.dt.float32


def _split_multi_waits(nc):
    """This walrus build encodes at most ONE sync-wait command per
    instruction. Hoist surplus waits onto standalone single-wait
    EventSemaphore instructions inserted just before the original."""
    ctr = 0
    for f in nc.m.functions:
        for bb in f.blocks:
            insts = bb.instructions
            i = 0
            while i < len(insts):
                inst = insts[i]
                si = inst.sync_info
                if si is not None and si.on_wait and len(si.on_wait) > 1:
                    waits = list(si.on_wait)
                    inst.sync_info = mybir.SyncInfo(
                        on_wait=[waits[-1]], on_update=list(si.on_update or [])
                    )
                    for w in waits[:-1]:
                        ev = mybir.InstEventSemaphore(
                            name=f"W-split-{ctr}", ins=[], outs=[]
                        )
                        ctr += 1
                        ev.engine = inst.engine
                        ev.sync_info = mybir.SyncInfo(on_wait=[w], on_update=[])
                        insts.insert(i, ev)
                        i += 1
                i += 1
    return ctr


def _build_nc(reps=1):
    nc = bass.Bass("TRN2", target_bir_lowering=False, debug=False)
    xt = nc.dram_tensor("xt", [D, S], f16, kind="ExternalInput").ap()
    wq = nc.dram_tensor("wq", [D, 128], f16, kind="ExternalInput").ap()
    wk = nc.dram_tensor("wk", [D, 128], f16, kind="ExternalInput").ap()
    wv = nc.dram_tensor("wv", [D, VW], f16, kind="ExternalInput").ap()
    bq = nc.dram_tensor("bq", [128, 1], f32, kind="ExternalInput").ap()
    bk = nc.dram_tensor("bk", [128, 1], f32, kind="ExternalInput").ap()
    bv = nc.dram_tensor("bv", [1, VW], f32, kind="ExternalInput").ap()
    out = nc.dram_tensor("out", [VW, S], f32, kind="ExternalOutput").ap()

    NDC = D // 128  # 4 contraction chunks

    def body(tc, rep, consts, xsb, qkv, ptp, outsb, wq_sb, wk_sb, wv_sb,
             bq_sb, bk_sb, bv_sb, shift_sb):
        # --- x^T load: 4 chunks of [128, S] ---
        x_sb = xsb.tile([128, NDC, S], f16, tag="x")
        xt_r = xt.rearrange("(c p) s -> p c s", p=128)
        for c in range(NDC):
            nc.sync.dma_start(out=x_sb[:, c, :], in_=xt_r[:, c, :])

        # --- projections ---
        qTd_sb = qkv.tile([128, S], f16, tag="qT")  # q^T dup'd on parts 64..127
        kTd_sb = qkv.tile([128, KC], f16, tag="kT")  # k^T (core's keys) dup'd
        v_sb = qkv.tile([128, NCHUNK * VW], f16, tag="v")  # [v|1] chunk-packed

        with tc.tile_pool(name=f"pproj{rep}", bufs=2, space="PSUM") as pproj:
            for m in range(NM):
                ps = pproj.tile([128, MT], f32, tag="pq")
                for c in range(NDC):
                    nc.tensor.matmul(
                        ps, lhsT=wq_sb[:, c, :], rhs=x_sb[:, c, m * MT:(m + 1) * MT],
                        start=(c == 0), stop=(c == NDC - 1),
                    )
                nc.vector.tensor_scalar_add(
                    out=qTd_sb[:, m * MT:(m + 1) * MT], in0=ps, scalar1=bq_sb
                )
            for n in range(KC // MT):
                ps = pproj.tile([128, MT], f32, tag="pq")
                for c in range(NDC):
                    nc.tensor.matmul(
                        ps, lhsT=wk_sb[:, c, :], rhs=x_sb[:, c, n * MT:(n + 1) * MT],
                        start=(c == 0), stop=(c == NDC - 1),
                    )
                nc.vector.tensor_scalar_add(
                    out=kTd_sb[:, n * MT:(n + 1) * MT], in0=ps, scalar1=bk_sb
                )
            for s_ in range(NCHUNK):
                ps = pproj.tile([128, VW], f32, tag="pv")
                for c in range(NDC):
                    nc.tensor.matmul(
                        ps, lhsT=x_sb[:, c, s_ * 128:(s_ + 1) * 128], rhs=wv_sb[:, c, :],
                        start=(c == 0), stop=(c == NDC - 1),
                    )
                nc.vector.tensor_add(
                    out=v_sb[:, s_ * VW:(s_ + 1) * VW], in0=ps, in1=bv_sb
                )

        # --- main attention loop ---
        with (
            tc.tile_pool(name=f"psc{rep}", bufs=3, space="PSUM") as pscp,
            tc.tile_pool(name=f"pout{rep}", bufs=2, space="PSUM") as poutp,
        ):
            for m in range(NM):
                ms = m * MT
                po = poutp.tile([VW, MT], f32, tag="po")
                pending = None  # software pipeline: attn mms lag scores by 1
                for j in range(NPAIR):
                    ca, cb = 2 * j, 2 * j + 1
                    psc = pscp.tile([128, 2 * MT], f32, tag="psc")
                    nc.tensor.matmul(
                        psc[:, 0:MT],
                        lhsT=kTd_sb[0:64, ca * 128:(ca + 1) * 128],
                        rhs=qTd_sb[0:64, ms:ms + MT],
                        start=True, stop=True,
                    )
                    nc.tensor.matmul(
                        psc[:, MT:2 * MT],
                        lhsT=kTd_sb[64:128, cb * 128:(cb + 1) * 128],
                        rhs=qTd_sb[64:128, ms:ms + MT],
                        start=True, stop=True,
                    )
                    pt = ptp.tile([128, 2 * MT], f16, tag="pt")
                    nc.scalar.activation(
                        out=pt, in_=psc, func=mybir.ActivationFunctionType.Exp,
                        bias=shift_sb, scale=0.125,
                    )
                    if pending is not None:
                        _emit_attn(nc, po, v_sb, pending[0], pending[1], NPAIR)
                    pending = (pt, j)
                _emit_attn(nc, po, v_sb, pending[0], pending[1], NPAIR)
                po_sb = outsb.tile([VW, MT], f32, tag="posb")
                nc.vector.tensor_copy(out=po_sb, in_=po)
                nc.sync.dma_start(out=out[:, ms:ms + MT], in_=po_sb)

    with tile.TileContext(nc) as tc:
        with (
            tc.tile_pool(name="consts", bufs=1) as consts,
            tc.tile_pool(name="xsb", bufs=1) as xsb,
            tc.tile_pool(name="qkv", bufs=1) as qkv,
            tc.tile_pool(name="pt", bufs=3) as ptp,
            tc.tile_pool(name="outsb", bufs=2) as outsb,
        ):
            # --- constants ---
            wq_sb = consts.tile([128, NDC, 128], f16)
            wk_sb = consts.tile([128, NDC, 128], f16)
            wv_sb = consts.tile([128, NDC, VW], f16)
            nc.sync.dma_start(out=wq_sb, in_=wq.rearrange("(c p) m -> p c m", p=128))
            nc.sync.dma_start(out=wk_sb, in_=wk.rearrange("(c p) m -> p c m", p=128))
            nc.sync.dma_start(out=wv_sb, in_=wv.rearrange("(c p) m -> p c m", p=128))
            bq_sb = consts.tile([128, 1], f32)
            bk_sb = consts.tile([128, 1], f32)
            bv_sb = consts.tile([128, VW], f32)
            nc.sync.dma_start(out=bq_sb, in_=bq)
            nc.sync.dma_start(out=bk_sb, in_=bk)
            bv_bcast = bass.AP(tensor=bv.tensor, offset=bv.offset, ap=[[0, 128], [1, VW]])
            nc.sync.dma_start(out=bv_sb, in_=bv_bcast)
            shift_sb = consts.tile([128, 1], f32)
            nc.vector.memset(shift_sb, EXP_SHIFT)

            for rep in range(reps):
                body(tc, rep, consts, xsb, qkv, ptp, outsb, wq_sb, wk_sb,
                     wv_sb, bq_sb, bk_sb, bv_sb, shift_sb)

    _split_multi_waits(nc)
    return nc


def _emit_attn(nc, po, v_sb, pt, j, npair):
    ca, cb = 2 * j, 2 * j + 1
    MT_ = pt.shape[-1] // 2
    nc.tensor.matmul(
        po, lhsT=v_sb[:, ca * VW:(ca + 1) * VW], rhs=pt[:, 0:MT_],
        start=(j == 0), stop=False,
    )
    nc.tensor.matmul(
        po, lhsT=v_sb[:, cb * VW:(cb + 1) * VW], rhs=pt[:, MT_:2 * MT_],
        start=False, stop=(j == npair - 1),
    )


_NC_CACHE = []


def _prepare_in_maps(x, Wq, bq, Wk, bk, Wv, bv):
    x = np.asarray(x, dtype=np.float32)
    Wq, Wk, Wv = (np.asarray(a, dtype=np.float32) for a in (Wq, Wk, Wv))
    bq, bk, bv = (np.asarray(a, dtype=np.float32) for a in (bq, bk, bv))

    wq_dup = np.concatenate([Wq, Wq], axis=1).astype(np.float16)      # [512,128]
    wk_dup = np.concatenate([Wk, Wk], axis=1).astype(np.float16)
    wv_aug = np.concatenate([Wv, np.zeros((D, 1), np.float32)], axis=1).astype(np.float16)
    bq_dup = np.concatenate([bq, bq]).astype(np.float32).reshape(128, 1)
    bk_dup = np.concatenate([bk, bk]).astype(np.float32).reshape(128, 1)
    bv_aug = np.concatenate([bv, np.ones(1, np.float32)]).astype(np.float32).reshape(1, VW)

    in_maps = []
    for c in range(N_CORES):
        b, h = c // 2, c % 2
        xt_b = np.ascontiguousarray(x[b].T)  # [512, 4096]
        if h == 1:
            xt_b = np.roll(xt_b, -KC, axis=1)  # core's key half first
        in_maps.append({
            "xt": xt_b.astype(np.float16),
            "wq": wq_dup, "wk": wk_dup, "wv": wv_aug,
            "bq": bq_dup, "bk": bk_dup, "bv": bv_aug,
        })
    return in_maps


def _merge_results(results):
    out = np.empty((B, S, H), dtype=np.float32)
    for b in range(B):
        a = results[2 * b]["out"].astype(np.float64)       # natural q order
        bb = results[2 * b + 1]["out"].astype(np.float64)  # q order rolled by -KC
        bb = np.roll(bb, KC, axis=1)                       # undo the roll
        tot = a + bb
        out[b] = (tot[:H, :] / tot[H:H + 1, :]).T.astype(np.float32)
    return out


def kernel(x, Wq, bq, Wk, bk, Wv, bv):
    in_maps = _prepare_in_maps(x, Wq, bq, Wk, bk, Wv, bv)
    if not _NC_CACHE:
        _NC_CACHE.append(_build_nc())
    nc = _NC_CACHE[0]
    res = bass_utils.run_bass_kernel_spmd(nc, in_maps, core_ids=list(range(N_CORES)))
    return _merge_results(res.results)
